# revision 1
# baseline (speedup 1.0000x reference)
"""Trainium2 Bass kernel for nn_Decoder_arch2 (LSTM image-caption decoder).

Reference computation (B=128, T=24 used steps, E=512, H2=1024, V=30000):
  tok = emb[captions]; seq = [pad_emb, tok[:, :23]]           # [B, 24, E]
  x_t = concat(seq_t, features)                               # [B, 2E]
  xg = x @ W_ih.T + b_ih + b_hh                               # [B, 24, 4096]
  24x LSTMCell steps (h = o*tanh(c), c = f*c + i*tanh(g))
  logits_t = h_t @ W_out.T + b_out                            # [B, 24, V]
  out = transpose(logits, (0, 2, 1))                          # [B, V, 24]
(The reference computes 25 steps and drops the last logit column, so step 25
and the last caption token are never needed.)

Sharding: pure data-parallel over batch. 8 cores x 16 batch rows each; every
core holds the full weights and computes its shard end-to-end. No collectives.

Device layouts (per core, partition dim always 128):
  gathered embeddings -> PE-transposed to xT[ec] [128(e), 384(t*16+b)] bf16
  xg_sb  [128, 24t, 32gc, 16b] fp32   (gate g = gc*128 + p)
  hs_sb  [128, 8hc, 24t, 16b] bf16    (hidden u = hc*128 + p)
  LSTM gates accumulate in one PSUM bank as [128, 32gc, 16b]
  projection: W_out tiles stationary, hs chunks moving, out [128(v), 384(t,b)]

Host pre-transposes/casts all weights (free layout prep) and reassembles the
[128, 30000, 24] output from the per-core [235, 128, 384] tensors.
"""

import sys

if "/opt/trn_rl_repo" not in sys.path:
    sys.path.insert(0, "/opt/trn_rl_repo")

import numpy as np
import ml_dtypes

import concourse.bass as bass
import concourse.bacc as bacc
import concourse.mybir as mybir
import concourse.tile as tile
from concourse.bass_utils import run_bass_kernel_spmd
from concourse.masks import make_identity

bf16 = ml_dtypes.bfloat16
F32 = mybir.dt.float32
BF16 = mybir.dt.bfloat16
I32 = mybir.dt.int32

B, T, E, V, H2 = 128, 24, 512, 30000, 1024
G = 4 * H2  # 4096
NC_N = 8
BS = B // NC_N  # 16 batch rows per core
NVT = 235  # ceil(30000/128)
VP = NVT * 128  # 30080
NCOL = T * BS  # 384 moving columns (t*16 + b)
STG = 4  # vt tiles per output staging DMA


def build_nc():
    nc = bacc.Bacc(None, target_bir_lowering=False)

    emb_d = nc.dram_tensor("embB", [V, E], BF16, kind="ExternalInput")
    idx_d = nc.dram_tensor("idx", [128, 3], I32, kind="ExternalInput")
    feat_d = nc.dram_tensor("featT", [128, 4, BS], BF16, kind="ExternalInput")
    wih_d = nc.dram_tensor("wihT", [8, 128, G], BF16, kind="ExternalInput")
    whh_d = nc.dram_tensor("whhT", [8, 128, G], BF16, kind="ExternalInput")
    bsum_d = nc.dram_tensor("bsum", [128, 32, BS], F32, kind="ExternalInput")
    bout_d = nc.dram_tensor("bout", [128, NVT], F32, kind="ExternalInput")
    wop_d = nc.dram_tensor("wop", [NVT, 128, H2], BF16, kind="ExternalInput")
    out_d = nc.dram_tensor("out", [NVT, 128, NCOL], F32, kind="ExternalOutput")

    with tile.TileContext(nc) as tc:
        with (
            tc.tile_pool(name="const", bufs=1) as const,
            tc.tile_pool(name="ge", bufs=3) as gep,
            tc.tile_pool(name="xt", bufs=4) as xtp,
            tc.tile_pool(name="w", bufs=8) as wp,
            tc.tile_pool(name="big", bufs=1) as big,
            tc.tile_pool(name="tmp", bufs=2) as tmp,
            tc.tile_pool(name="wout", bufs=12) as woutp,
            tc.tile_pool(name="stage", bufs=3) as stagep,
            tc.tile_pool(name="pm", bufs=2, space="PSUM") as pmp,
            tc.tile_pool(name="pf", bufs=1, space="PSUM") as pfp,
            tc.tile_pool(name="pg", bufs=2, space="PSUM") as pgp,
            tc.tile_pool(name="po", bufs=3, space="PSUM") as pop,
        ):
            # ---- constants / small inputs ----
            idx_sb = const.tile([128, 3], I32)
            nc.sync.dma_start(idx_sb[:], idx_d[:])
            feat_sb = const.tile([128, 4, BS], BF16)
            nc.sync.dma_start(feat_sb[:], feat_d[:])
            bsum_sb = const.tile([128, 32, BS], F32)
            nc.sync.dma_start(bsum_sb[:], bsum_d[:])
            bout_sb = const.tile([128, NVT], F32)
            nc.sync.dma_start(bout_sb[:], bout_d[:])
            ident = const.tile([128, 128], BF16)
            make_identity(nc, ident)

            # ---- W_ih tiles (4 seq + 4 feat), then W_hh reuses the slots ----
            wih_seq = []
            for ec in range(4):
                t_ = wp.tile([128, G], BF16, tag="w")
                nc.sync.dma_start(t_[:], wih_d[ec])
                wih_seq.append(t_)
            wih_feat = []
            for ec in range(4):
                t_ = wp.tile([128, G], BF16, tag="w")
                nc.sync.dma_start(t_[:], wih_d[4 + ec])
                wih_feat.append(t_)

            # ---- embedding gather + transpose into xT ----
            ge = []
            for r in range(3):
                g_t = gep.tile([128, E], BF16)
                nc.gpsimd.indirect_dma_start(
                    out=g_t[:],
                    out_offset=None,
                    in_=emb_d[:],
                    in_offset=bass.IndirectOffsetOnAxis(ap=idx_sb[:, r : r + 1], axis=0),
                )
                ge.append(g_t)

            xt = [xtp.tile([128, NCOL], BF16, tag="xt", name=f"xt{i}") for i in range(4)]
            for ec in range(4):
                for r in range(3):
                    pt = pmp.tile([128, 128], BF16, tag="pm")
                    nc.tensor.transpose(
                        pt[:], ge[r][:, ec * 128 : (ec + 1) * 128], ident[:]
                    )
                    nc.vector.tensor_copy(
                        xt[ec][:, r * 128 : (r + 1) * 128], pt[:]
                    )

            # ---- feature-side gate projection fg = W_ih[:, E:] @ feat + bsum ----
            psum_fg = pfp.tile([128, 32, BS], F32)
            for gc in range(32):
                for ec in range(4):
                    nc.tensor.matmul(
                        psum_fg[:, gc, :],
                        wih_feat[ec][:, gc * 128 : (gc + 1) * 128],
                        feat_sb[:, ec, :],
                        start=(ec == 0),
                        stop=(ec == 3),
                    )
            fg_sb = big.tile([128, 32, BS], F32, tag="fg")
            nc.vector.tensor_add(fg_sb[:], psum_fg[:], bsum_sb[:])

            # ---- xg GEMM (token side): xg[g, (t,b)] = W_ih[:, :E] @ seq ----
            xg_sb = big.tile([128, T, 32, BS], F32, tag="xg")
            for gc in range(32):
                psum_xg = pmp.tile([128, T, BS], F32, tag="pm")
                for ec in range(4):
                    nc.tensor.matmul(
                        psum_xg[:],
                        wih_seq[ec][:, gc * 128 : (gc + 1) * 128],
                        xt[ec][:],
                        start=(ec == 0),
                        stop=(ec == 3),
                    )
                nc.scalar.activation(
                    xg_sb[:, :, gc, :], psum_xg[:], mybir.ActivationFunctionType.Copy
                )
            # add fg (and bias) for every timestep
            for t in range(T):
                nc.vector.tensor_add(xg_sb[:, t], xg_sb[:, t], fg_sb[:])

            # ---- W_hh tiles (reuse the 8 "w" slots) ----
            whh = []
            for hc in range(8):
                t_ = wp.tile([128, G], BF16, tag="w")
                nc.sync.dma_start(t_[:], whh_d[hc])
                whh.append(t_)

            # ---- LSTM ----
            hs_sb = big.tile([128, 8, T, BS], BF16, tag="hs")
            c_sb = big.tile([128, 8, BS], F32, tag="c")
            SIG = mybir.ActivationFunctionType.Sigmoid
            TANH = mybir.ActivationFunctionType.Tanh

            for t in range(T):
                if t == 0:
                    gsrc = xg_sb[:, 0]  # [128, 32, BS], c=0, h=0
                else:
                    pg_t = pgp.tile([128, 32, BS], F32, tag="pg")
                    for gc in range(32):
                        for hc in range(8):
                            nc.tensor.matmul(
                                pg_t[:, gc, :],
                                whh[hc][:, gc * 128 : (gc + 1) * 128],
                                hs_sb[:, hc, t - 1, :],
                                start=(hc == 0),
                                stop=(hc == 7),
                            )
                    nc.vector.tensor_add(pg_t[:], pg_t[:], xg_sb[:, t])
                    gsrc = pg_t

                t_i = tmp.tile([128, 8, BS], F32, tag="ti")
                t_f = tmp.tile([128, 8, BS], F32, tag="tf", name="t_f") if t > 0 else None
                t_g = tmp.tile([128, 8, BS], F32, tag="tg")
                t_o = tmp.tile([128, 8, BS], F32, tag="to")
                t_c = tmp.tile([128, 8, BS], F32, tag="tc")
                nc.scalar.activation(t_i[:], gsrc[:, 0:8, :], SIG)
                nc.scalar.activation(t_g[:], gsrc[:, 16:24, :], TANH)
                nc.scalar.activation(t_o[:], gsrc[:, 24:32, :], SIG)
                if t == 0:
                    nc.vector.tensor_mul(c_sb[:], t_i[:], t_g[:])
                else:
                    nc.scalar.activation(t_f[:], gsrc[:, 8:16, :], SIG)
                    nc.vector.tensor_mul(t_f[:], t_f[:], c_sb[:])
                    nc.vector.tensor_mul(t_i[:], t_i[:], t_g[:])
                    nc.vector.tensor_add(c_sb[:], t_f[:], t_i[:])
                nc.scalar.activation(t_c[:], c_sb[:], TANH)
                nc.vector.tensor_mul(hs_sb[:, :, t, :], t_o[:], t_c[:])

            # ---- output projection ----
            stage_t = None
            for vt in range(NVT):
                w_t = woutp.tile([128, H2], BF16, tag="wo")
                nc.sync.dma_start(w_t[:], wop_d[vt])
                po_t = pop.tile([128, T, BS], F32, tag="po")
                for hc in range(8):
                    nc.tensor.matmul(
                        po_t[:],
                        w_t[:, hc * 128 : (hc + 1) * 128],
                        hs_sb[:, hc],
                        start=(hc == 0),
                        stop=(hc == 7),
                    )
                sj = vt % STG
                if sj == 0:
                    stage_t = stagep.tile([128, STG, T, BS], F32, tag="st")
                nc.scalar.activation(
                    stage_t[:, sj],
                    po_t[:],
                    mybir.ActivationFunctionType.Identity,
                    bias=bout_sb[:, vt : vt + 1],
                )
                if sj == STG - 1 or vt == NVT - 1:
                    nv = sj + 1
                    dst = out_d[vt - sj : vt + 1].rearrange("j p c -> p j c")
                    src = stage_t[:, :nv].rearrange("p j t b -> p j (t b)")
                    nc.sync.dma_start(dst, src)

    nc.compile()
    return nc


def prep_host(features, captions, pad_idx, emb, W_ih, W_hh, b_ih, b_hh, W_out, b_out):
    """Host-side layout prep. Returns (shared dict, per-core list of dicts)."""
    from einops import rearrange

    features = np.asarray(features, dtype=np.float32)
    captions = np.asarray(captions).astype(np.int64)
    pad_idx = int(np.asarray(pad_idx))
    emb = np.asarray(emb, dtype=np.float32)
    W_ih = np.asarray(W_ih, dtype=np.float32)
    W_hh = np.asarray(W_hh, dtype=np.float32)
    b_ih = np.asarray(b_ih, dtype=np.float32)
    b_hh = np.asarray(b_hh, dtype=np.float32)
    W_out = np.asarray(W_out, dtype=np.float32)
    b_out = np.asarray(b_out, dtype=np.float32)

    embB = np.ascontiguousarray(emb.astype(bf16))
    wihT = np.ascontiguousarray(rearrange(W_ih, "g (kc p) -> kc p g", p=128).astype(bf16))
    whhT = np.ascontiguousarray(rearrange(W_hh, "g (hc p) -> hc p g", p=128).astype(bf16))
    bsum = rearrange(b_ih + b_hh, "(gc p) -> p gc", p=128).astype(np.float32)
    bsum = np.ascontiguousarray(np.repeat(bsum[:, :, None], BS, axis=2))

    W_out_p = np.zeros((VP, H2), np.float32)
    W_out_p[:V] = W_out
    wop = np.ascontiguousarray(
        rearrange(W_out_p, "(vt f) (hc p) -> vt p (hc f)", f=128, p=128).astype(bf16)
    )
    b_out_p = np.zeros((VP,), np.float32)
    b_out_p[:V] = b_out
    bout = np.ascontiguousarray(rearrange(b_out_p, "(vt p) -> p vt", p=128))

    shared = {"embB": embB, "wihT": wihT, "whhT": whhT, "bsum": bsum,
              "wop": wop, "bout": bout}

    per_core = []
    for c in range(NC_N):
        bsl = slice(c * BS, (c + 1) * BS)
        gidx = np.zeros((T, BS), np.int64)  # row r = t*BS + b
        gidx[0, :] = pad_idx
        gidx[1:, :] = captions[bsl, : T - 1].T
        idx = np.ascontiguousarray(
            gidx.reshape(3, 128).T.astype(np.int32)
        )  # [128, 3]: idx[p, r3] = gidx_flat[r3*128 + p]
        featT = np.ascontiguousarray(
            rearrange(features[bsl], "b (ec p) -> p ec b", p=128).astype(bf16)
        )
        per_core.append({"idx": idx, "featT": featT})
    return shared, per_core


_NC_CACHE = None


def kernel(**inputs) -> np.ndarray:
    global _NC_CACHE
    if _NC_CACHE is None:
        _NC_CACHE = build_nc()
    nc = _NC_CACHE

    shared, per_core = prep_host(**inputs)
    in_maps = [dict(shared, **pc) for pc in per_core]
    res = run_bass_kernel_spmd(nc, in_maps, core_ids=list(range(NC_N)))

    out = np.empty((B, V, T), np.float32)
    for c in range(NC_N):
        o = res.results[c]["out"]  # [NVT, 128, NCOL], col = t*BS + b
        o = o.reshape(NVT, 128, T, BS)
        o = o.transpose(3, 0, 1, 2).reshape(BS, VP, T)
        out[c * BS : (c + 1) * BS] = o[:, :V, :]
    return out



# revision 2
# speedup vs baseline: 1.5023x; 1.5023x over previous
"""Trainium2 Bass kernel v2 for nn_Decoder_arch2 (LSTM image-caption decoder).

Reference computation (B=128, T=24 used steps, E=512, H2=1024, V=30000):
  tok = emb[captions]; seq = [pad_emb, tok[:, :23]]           # [B, 24, E]
  x_t = concat(seq_t, features)                               # [B, 2E]
  xg = x @ W_ih.T + b_ih + b_hh                               # [B, 24, 4096]
  24x LSTMCell steps (c = f*c + i*tanh(g); h = o*tanh(c))
  logits_t = h_t @ W_out.T + b_out                            # [B, 24, V]
  out = transpose(logits, (0, 2, 1))                          # [B, V, 24]

Sharding: batch-parallel LSTM (16 rows/core) + vocab-parallel projection
(3840 vocab rows/core) bridged by a chunked AllGather of the hidden states.
All pools coexist in one scope so projection matmuls fill PE gaps during the
LSTM as soon as each AllGather chunk lands.

Per-core phases:
  1. xg GEMM in transposed orientation: stationary = x.T blocks packed as
     [e-chunk, 8t x 16b], moving = W_ih.T (streamed in 512-col chunks) ->
     xgT [128(t,b), 3 tblk, 4096], bias folded in via a K=1 ones-row matmul.
     W_ih/bias pre-scaled by 2^14 so the fp8 recurrence shares one descale.
  2. LSTM in transposed orientation: per step, gates.T [16b, 4096g] accumulate
     in PSUM octants [16, 512]: one identity-matmul injects the xgT slice,
     then 4 fp8 DoubleRow matmuls (h chunk-pairs stationary, W_hh.T moving)
     add the recurrent term. W_hh is pre-scaled x256 and h x64 to keep fp8
     e4m3 out of its subnormal range; the gate activations descale by 2^-14.
     h_t is transposed back to [128(hu), 16b] via 8 PE transposes feeding the
     next step's stationaries (fp8) and the hs buffer (bf16).
  3. AllGather of hs in 6 t-chunks (every 4 steps) across the 8 cores.
  4. Projection in 6 t-phases: phase ph needs only AllGather chunk ph, so
     most phases overlap the LSTM. W_out tiles streamed per (phase, vt).

Host does all layout prep (gather, transposes, casts, scaling) and output
assembly.
"""

import sys

if "/opt/trn_rl_repo" not in sys.path:
    sys.path.insert(0, "/opt/trn_rl_repo")

import numpy as np
import ml_dtypes

import concourse.bass as bass
import concourse.bacc as bacc
import concourse.mybir as mybir
import concourse.tile as tile
from concourse.bass_utils import run_bass_kernel_spmd

bf16 = ml_dtypes.bfloat16
fp8 = ml_dtypes.float8_e4m3
F32 = mybir.dt.float32
BF16 = mybir.dt.bfloat16
FP8 = mybir.dt.float8e4

B, T, E, V, H2 = 128, 24, 512, 30000, 1024
G = 4 * H2  # 4096
NC_N = 8
BS = B // NC_N  # 16 batch rows per core
TB = 3  # t-blocks of 8 timesteps (8t x 16b = 128 partitions)
KC = 6  # AllGather / projection chunks of TPC timesteps
TPC = T // KC  # 4
NVT = 240  # padded vocab tiles total (30720 rows)
VP = NVT * 128
NVT_C = NVT // NC_N  # 30 vt tiles per core
VSH = NVT_C * 128  # 3840 vocab rows per core

WSC = 256.0  # W_hh fp8 pre-scale
HSC = 64.0  # h fp8 pre-scale
SC = WSC * HSC  # total gate pre-activation scale (W_ih/bias pre-scaled by SC)

SIG = mybir.ActivationFunctionType.Sigmoid
TANH = mybir.ActivationFunctionType.Tanh
COPY = mybir.ActivationFunctionType.Copy
IDENT = mybir.ActivationFunctionType.Identity
DR = mybir.MatmulPerfMode.DoubleRow


def build_nc():
    nc = bacc.Bacc(None, target_bir_lowering=False)

    xstat_d = nc.dram_tensor("xstat", [128, TB, 4, 128], BF16, kind="ExternalInput")
    featst_d = nc.dram_tensor("featst", [128, 4, 128], BF16, kind="ExternalInput")
    wihT_d = nc.dram_tensor("wihT", [128, 8, G], BF16, kind="ExternalInput")
    whh8_d = nc.dram_tensor("whh8", [128, 4, 2, G], FP8, kind="ExternalInput")
    brow_d = nc.dram_tensor("brow", [1, G], BF16, kind="ExternalInput")
    ones_d = nc.dram_tensor("onesrow", [1, 128], BF16, kind="ExternalInput")
    id16_d = nc.dram_tensor("ident16", [16, 16], BF16, kind="ExternalInput")
    wop_d = nc.dram_tensor("wop", [NVT_C, 128, 8, 128], BF16, kind="ExternalInput")
    boutT_d = nc.dram_tensor("boutT", [128, NVT_C], F32, kind="ExternalInput")
    # one output tensor per t-phase of TPC steps; cols = (r, t_in_phase, b)
    out_d = [
        nc.dram_tensor(f"out{ph}", [NVT_C, 128, NC_N, TPC, BS], BF16,
                       kind="ExternalOutput")
        for ph in range(KC)
    ]

    with tile.TileContext(nc) as tc:
        with (
            tc.tile_pool(name="const", bufs=1) as const,
            tc.tile_pool(name="hs", bufs=1) as hsp,
            tc.tile_pool(name="hsall", bufs=4) as hap,
            tc.tile_pool(name="xg", bufs=1) as xgp,
            tc.tile_pool(name="wih", bufs=1) as wihp,
            tc.tile_pool(name="whh", bufs=1) as whhp,
            tc.tile_pool(name="cst", bufs=1) as cstp,
            tc.tile_pool(name="tmp", bufs=1) as tmp,
            tc.tile_pool(name="h8", bufs=2) as h8p,
            tc.tile_pool(name="xgt", bufs=1) as xgtp,
            tc.tile_pool(name="wo", bufs=1) as wo_p,
            tc.tile_pool(name="stg", bufs=2) as stgp,
            tc.tile_pool(name="dram", bufs=1, space="DRAM") as dram,
            tc.tile_pool(name="ps", bufs=5, space="PSUM") as psp,
            tc.tile_pool(name="po", bufs=2, space="PSUM") as pop,
            tc.tile_pool(name="ptr", bufs=1, space="PSUM") as ptrp,
        ):
            xstat_sb = const.tile([128, TB, 4, 128], BF16)
            nc.sync.dma_start(xstat_sb[:], xstat_d[:])
            featst_sb = const.tile([128, 4, 128], BF16)
            nc.sync.dma_start(featst_sb[:], featst_d[:])
            brow_sb = const.tile([1, G], BF16)
            nc.sync.dma_start(brow_sb[:], brow_d[:])
            ones_sb = const.tile([1, 128], BF16)
            nc.sync.dma_start(ones_sb[:], ones_d[:])
            id16_sb = const.tile([16, 16], BF16)
            nc.sync.dma_start(id16_sb[:], id16_d[:])
            boutT_sb = const.tile([128, NVT_C], F32)
            nc.sync.dma_start(boutT_sb[:], boutT_d[:])

            hs_sb = hsp.tile([128, 8, T, BS], BF16)  # [hu, hc, t, b]
            xgT = xgp.tile([128, TB, G], BF16)  # [8t*16b, tblk, g] (x SC)
            whh_sb = whhp.tile([128, 4, 2, G], FP8)
            nc.sync.dma_start(whh_sb[:], whh8_d[:])
            c_sb = cstp.tile([16, H2], F32)
            # all W_out tiles as one resident tile, loaded once on the
            # scalar ring so the sync ring stays free for xg staging
            wop_all = wo_p.tile([128, NVT_C, 8, 128], BF16)
            nc.scalar.dma_start(
                wop_all[:], wop_d[:].rearrange("vt p hc j -> p vt hc j")
            )
            hs_all_t = []  # per-AG-chunk gathered hs tiles (ring of 4)
            ag_in = [
                dram.tile([128, 8, TPC, BS], BF16, name=f"agi{k}") for k in range(KC)
            ]
            ag_out = [
                dram.tile([NC_N * 128, 8, TPC, BS], BF16, addr_space="Shared",
                          name=f"ago{k}")
                for k in range(KC)
            ]

            # ---- phase 1: xg GEMM (transposed orientation), W_ih streamed ----
            for cc in range(8):
                ccs = slice(cc * 512, (cc + 1) * 512)
                wih_c = wihp.tile([128, 8, 512], BF16, tag="wih")
                nc.sync.dma_start(wih_c[:], wihT_d[:, :, ccs])
                for tblk in range(TB):
                    px = psp.tile([128, 512], F32, tag="ps")
                    for ec in range(4):
                        nc.tensor.matmul(
                            px[:],
                            xstat_sb[:, tblk, ec],
                            wih_c[:, ec, :],
                            start=(ec == 0),
                            stop=False,
                        )
                    for ec in range(4):
                        nc.tensor.matmul(
                            px[:],
                            featst_sb[:, ec],
                            wih_c[:, 4 + ec, :],
                            start=False,
                            stop=False,
                        )
                    nc.tensor.matmul(
                        px[:], ones_sb[:], brow_sb[:, ccs], start=False, stop=True
                    )
                    nc.scalar.activation(xgT[:, tblk, ccs], px[:], COPY)

            # ---- phase 2: LSTM (transposed orientation, fp8 recurrence) ----
            # octant oc covers gate columns [oc*512, (oc+1)*512);
            # gate quarters: q0=i, q1=f, q2=g, q3=o. Emit o, f, i, g.
            OC_ORDER = [6, 7, 2, 3, 0, 1, 4, 5]
            for t in range(T):
                tblk, p0 = t // 8, (t % 8) * BS
                # stage this step's xg slice down to partition base 0
                # (PE operands require base partition 0/32/64)
                xg_t = xgtp.tile([16, G], BF16, tag="xgt")
                nc.sync.dma_start(xg_t[:], xgT[p0 : p0 + BS, tblk, :])

                t_i = tmp.tile([16, H2], F32, tag="ti")
                t_f = tmp.tile([16, H2], F32, tag="tf")
                t_g = tmp.tile([16, H2], F32, tag="tg")
                t_o = tmp.tile([16, H2], F32, tag="to")
                gate_tmp = {0: t_i, 1: t_f, 2: t_g, 3: t_o}

                for oc in OC_ORDER:
                    if t == 0 and oc in (2, 3):
                        continue  # f unused at t=0 (c_0 = 0)
                    q, half = oc // 2, oc % 2
                    cs = slice(oc * 512, (oc + 1) * 512)
                    po_ = psp.tile([16, 512], F32, tag="ps")
                    nc.tensor.matmul(
                        po_[:], id16_sb[:], xg_t[:, cs],
                        start=True, stop=(t == 0),
                    )
                    if t > 0:
                        for j in range(4):
                            nc.tensor.matmul(
                                po_[:],
                                h8_prev[:, 2 * j : 2 * j + 2, :],
                                whh_sb[:, j, :, cs],
                                start=False,
                                stop=(j == 3),
                                perf_mode=DR,
                            )
                    func = TANH if q == 2 else SIG
                    dst = gate_tmp[q]
                    nc.scalar.activation(
                        dst[:, half * 512 : (half + 1) * 512], po_[:],
                        func, scale=1.0 / SC,
                    )

                if t == 0:
                    nc.vector.tensor_mul(c_sb[:], t_i[:], t_g[:])
                else:
                    nc.vector.tensor_mul(t_f[:], t_f[:], c_sb[:])
                    nc.vector.tensor_mul(t_i[:], t_i[:], t_g[:])
                    nc.vector.tensor_add(c_sb[:], t_f[:], t_i[:])
                t_c = tmp.tile([16, H2], F32, tag="tg")  # reuse t_g's slot
                nc.scalar.activation(t_c[:], c_sb[:], TANH)
                hT = tmp.tile([16, H2], BF16, tag="tf")  # reuse t_f's slot
                nc.vector.tensor_mul(hT[:], t_o[:], t_c[:])

                ptr = ptrp.tile([128, 8, BS], BF16, tag="ptr")
                for hc in range(8):
                    nc.tensor.transpose(
                        ptr[:, hc], hT[:, hc * 128 : (hc + 1) * 128], id16_sb[:]
                    )
                nc.vector.tensor_copy(hs_sb[:, :, t, :], ptr[:])
                h8_prev = h8p.tile([128, 8, BS], FP8, tag="h8")
                nc.scalar.activation(h8_prev[:], ptr[:], COPY, scale=HSC)

                # ---- phase 3 (interleaved): chunked AllGather of hs ----
                if t % TPC == TPC - 1:
                    k = t // TPC
                    ts = slice(k * TPC, (k + 1) * TPC)
                    nc.gpsimd.dma_start(out=ag_in[k][:], in_=hs_sb[:, :, ts, :])
                    nc.gpsimd.collective_compute(
                        "AllGather",
                        mybir.AluOpType.bypass,
                        replica_groups=[list(range(NC_N))],
                        ins=[ag_in[k].opt()],
                        outs=[ag_out[k].opt()],
                    )
                    hs_k = hap.tile([128, 8, NC_N, TPC, BS], BF16, tag="ha")
                    hs_all_t.append(hs_k)
                    nc.gpsimd.dma_start(
                        out=hs_k[:],
                        in_=ag_out[k][:].rearrange(
                            "(r p) hc t b -> p hc r t b", p=128
                        ),
                    )

            # ---- phase 4: vocab-sharded projection, pipelined per t-phase ----
            # t-phase ph only needs AllGather chunk ph, so early phases overlap
            # the LSTM; W_out tiles are re-streamed per (phase, vt).
            for ph in range(KC):
                for vt in range(NVT_C):
                    po = pop.tile([128, NC_N, TPC, BS], F32, tag="po")
                    for hc in range(8):
                        nc.tensor.matmul(
                            po[:],
                            wop_all[:, vt, hc],
                            hs_all_t[ph][:, hc],
                            start=(hc == 0),
                            stop=(hc == 7),
                        )
                    st = stgp.tile([128, NC_N, TPC, BS], BF16, tag="st")
                    nc.scalar.activation(
                        st[:], po[:], IDENT, bias=boutT_sb[:, vt : vt + 1]
                    )
                    eng = nc.scalar if ph < 3 else nc.sync
                    eng.dma_start(out_d[ph][vt], st[:])

    nc.compile()
    return nc


def prep_host(features, captions, pad_idx, emb, W_ih, W_hh, b_ih, b_hh, W_out, b_out):
    """Host-side layout prep. Returns (shared dict, per-core list of dicts)."""
    features = np.asarray(features, dtype=np.float32)
    captions = np.asarray(captions).astype(np.int64)
    pad_idx = int(np.asarray(pad_idx))
    emb = np.asarray(emb, dtype=np.float32)
    W_ih = np.asarray(W_ih, dtype=np.float32)
    W_hh = np.asarray(W_hh, dtype=np.float32)
    bsum = np.asarray(b_ih, dtype=np.float32) + np.asarray(b_hh, dtype=np.float32)
    W_out = np.asarray(W_out, dtype=np.float32)
    b_out = np.asarray(b_out, dtype=np.float32)

    # seqtok[t, b]: pad for t=0, captions[b, t-1] for t>=1
    seqtok = np.empty((T, B), np.int64)
    seqtok[0, :] = pad_idx
    seqtok[1:, :] = captions[:, : T - 1].T
    xtok = emb[seqtok]  # [T, B, E]

    # wihT[p, ec, g] = W_ih[g, ec*128+p] * SC
    wihT = np.ascontiguousarray(
        (W_ih * SC).T.reshape(8, 128, G).transpose(1, 0, 2).astype(bf16)
    )
    # whh8[p, j, kt, g] = W_hh[g, (2j+kt)*128+p] * WSC
    whh8 = np.ascontiguousarray(
        (W_hh * WSC).T.reshape(4, 2, 128, G).transpose(2, 0, 1, 3).astype(fp8)
    )
    brow = np.ascontiguousarray((bsum * SC)[None, :].astype(bf16))
    onesrow = np.ones((1, 128), bf16)
    ident16 = np.eye(16, dtype=bf16)

    Wout_pad = np.zeros((VP, H2), np.float32)
    Wout_pad[:V] = W_out
    bout_pad = np.zeros((VP,), np.float32)
    bout_pad[:V] = b_out

    shared = {"wihT": wihT, "whh8": whh8, "brow": brow, "onesrow": onesrow,
              "ident16": ident16}

    per_core = []
    for c in range(NC_N):
        bsl = slice(c * BS, (c + 1) * BS)
        # xstat[p, tblk, ec, ti*16+bl] = xtok[tblk*8+ti, c*16+bl, ec*128+p]
        xs = xtok[:, bsl, :]  # [24, 16, 512]
        xs = xs.reshape(TB, 8, BS, 4, 128)  # [tblk, ti, bl, ec, p]
        xstat = np.ascontiguousarray(
            xs.transpose(4, 0, 3, 1, 2).reshape(128, TB, 4, 8 * BS).astype(bf16)
        )
        # featst[p, ec, ti*16+bl] = features[c*16+bl, ec*128+p]
        f = features[bsl].reshape(BS, 4, 128)  # [bl, ec, p]
        featst = np.ascontiguousarray(
            np.broadcast_to(
                f.transpose(2, 1, 0)[:, :, None, :], (128, 4, 8, BS)
            ).reshape(128, 4, 128).astype(bf16)
        )
        # wop[vt, p(hu), hc, j] = Wout_pad[c*3840 + vt*128 + j, hc*128 + p]
        w = Wout_pad[c * VSH : (c + 1) * VSH].reshape(NVT_C, 128, 8, 128)
        wop = np.ascontiguousarray(w.transpose(0, 3, 2, 1).astype(bf16))
        # boutT[p, vt] = bout_pad[c*3840 + vt*128 + p]
        bT = np.ascontiguousarray(
            bout_pad[c * VSH : (c + 1) * VSH].reshape(NVT_C, 128).T
        )
        per_core.append({"xstat": xstat, "featst": featst, "wop": wop, "boutT": bT})
    return shared, per_core


_NC_CACHE = None


def kernel(**inputs) -> np.ndarray:
    global _NC_CACHE
    if _NC_CACHE is None:
        _NC_CACHE = build_nc()
    nc = _NC_CACHE

    shared, per_core = prep_host(**inputs)
    in_maps = [dict(shared, **pc) for pc in per_core]
    res = run_bass_kernel_spmd(nc, in_maps, core_ids=list(range(NC_N)))

    out = np.empty((B, VP, T), np.float32)
    for c in range(NC_N):
        for ph in range(KC):
            o = np.asarray(res.results[c][f"out{ph}"])  # [30, 128, 8r, TPCt, 16b]
            a = o.astype(np.float32).transpose(2, 4, 0, 1, 3)  # [r, bl, vt, j, tp]
            out[:, c * VSH : (c + 1) * VSH, ph * TPC : (ph + 1) * TPC] = a.reshape(
                B, VSH, TPC
            )
    return out[:, :V, :]


# revision 4
# speedup vs baseline: 2.8140x; 1.8731x over previous
"""Trainium2 Bass kernel v2 for nn_Decoder_arch2 (LSTM image-caption decoder).

Reference computation (B=128, T=24 used steps, E=512, H2=1024, V=30000):
  tok = emb[captions]; seq = [pad_emb, tok[:, :23]]           # [B, 24, E]
  x_t = concat(seq_t, features)                               # [B, 2E]
  xg = x @ W_ih.T + b_ih + b_hh                               # [B, 24, 4096]
  24x LSTMCell steps (c = f*c + i*tanh(g); h = o*tanh(c))
  logits_t = h_t @ W_out.T + b_out                            # [B, 24, V]
  out = transpose(logits, (0, 2, 1))                          # [B, V, 24]

Sharding: batch-parallel LSTM (16 rows/core) + vocab-parallel projection
(3840 vocab rows/core) bridged by a chunked AllGather of the hidden states.
All pools coexist in one scope so projection matmuls fill PE gaps during the
LSTM as soon as each AllGather chunk lands.

Per-core phases:
  1. xg GEMM in transposed orientation: stationary = x.T blocks packed as
     [e-chunk, 8t x 16b], moving = W_ih.T (streamed in 512-col chunks) ->
     xgT [128(t,b), 3 tblk, 4096], bias folded in via a K=1 ones-row matmul.
     W_ih/bias pre-scaled by 2^14 so the fp8 recurrence shares one descale.
  2. LSTM in transposed orientation: per step, gates.T [16b, 4096g] accumulate
     in PSUM octants [16, 512]: one identity-matmul injects the xgT slice,
     then 4 fp8 DoubleRow matmuls (h chunk-pairs stationary, W_hh.T moving)
     add the recurrent term. W_hh is pre-scaled x256 and h x64 to keep fp8
     e4m3 out of its subnormal range; the gate activations descale by 2^-14.
     h_t is transposed back to [128(hu), 16b] via 8 PE transposes feeding the
     next step's stationaries (fp8) and the hs buffer (bf16).
  3. AllGather of hs in 6 t-chunks (every 4 steps) across the 8 cores.
  4. Projection in 6 t-phases: phase ph needs only AllGather chunk ph, so
     most phases overlap the LSTM. W_out tiles streamed per (phase, vt).

Host does all layout prep (gather, transposes, casts, scaling) and output
assembly.
"""

import sys

if "/opt/trn_rl_repo" not in sys.path:
    sys.path.insert(0, "/opt/trn_rl_repo")

import numpy as np
import ml_dtypes

import concourse.bass as bass
import concourse.bacc as bacc
import concourse.mybir as mybir
import concourse.tile as tile
from concourse.bass_utils import run_bass_kernel_spmd

bf16 = ml_dtypes.bfloat16
fp8 = ml_dtypes.float8_e4m3
F32 = mybir.dt.float32
BF16 = mybir.dt.bfloat16
FP8 = mybir.dt.float8e4

B, T, E, V, H2 = 128, 24, 512, 30000, 1024
G = 4 * H2  # 4096
NC_N = 8
BS = B // NC_N  # 16 batch rows per core
TB = 3  # t-blocks of 8 timesteps (8t x 16b = 128 partitions)
KC = 6  # AllGather / projection chunks of TPC timesteps
TPC = T // KC  # 4
NVT = 240  # padded vocab tiles total (30720 rows)
VP = NVT * 128
NVT_C = NVT // NC_N  # 30 vt tiles per core
VSH = NVT_C * 128  # 3840 vocab rows per core

WSC = 256.0  # W_hh fp8 pre-scale
HSC = 64.0  # h fp8 pre-scale
SC = WSC * HSC  # total gate pre-activation scale (W_ih/bias pre-scaled by SC)

SIG = mybir.ActivationFunctionType.Sigmoid
TANH = mybir.ActivationFunctionType.Tanh
COPY = mybir.ActivationFunctionType.Copy
IDENT = mybir.ActivationFunctionType.Identity
DR = mybir.MatmulPerfMode.DoubleRow


def _emit_quarter(nc, po_, id16_sb, xg_t, h8_prev, whh_sb, q, t):
    """Emit one gate quarter: per 512-col half (one PSUM bank), an identity
    matmul injecting the xg slice plus 4 fp8 DoubleRow recurrent matmuls."""
    for half in range(2):
        cs = slice(q * 1024 + half * 512, q * 1024 + (half + 1) * 512)
        hs_ = slice(half * 512, (half + 1) * 512)
        nc.tensor.matmul(
            po_[:, hs_], id16_sb[:], xg_t[:, cs], start=True, stop=(t == 0)
        )
        if t == 0:
            continue
        for j in range(4):
            nc.tensor.matmul(
                po_[:, hs_],
                h8_prev[:, 2 * j : 2 * j + 2, :],
                whh_sb[:, j, :, cs],
                start=False,
                stop=(j == 3),
                perf_mode=DR,
            )


def build_nc():
    nc = bacc.Bacc(None, target_bir_lowering=False)

    xstat_d = nc.dram_tensor("xstat", [128, TB, 4, 128], BF16, kind="ExternalInput")
    featst_d = nc.dram_tensor("featst", [128, 4, 128], BF16, kind="ExternalInput")
    wihT_d = nc.dram_tensor("wihT", [128, 8, G], BF16, kind="ExternalInput")
    whh8_d = nc.dram_tensor("whh8", [128, 4, 2, G], FP8, kind="ExternalInput")
    brow_d = nc.dram_tensor("brow", [1, G], BF16, kind="ExternalInput")
    ones_d = nc.dram_tensor("onesrow", [1, 128], BF16, kind="ExternalInput")
    id16_d = nc.dram_tensor("ident16", [16, 16], BF16, kind="ExternalInput")
    wop_d = nc.dram_tensor("wop", [NVT_C, 128, 8, 128], BF16, kind="ExternalInput")
    boutT_d = nc.dram_tensor("boutT", [128, NVT_C], F32, kind="ExternalInput")
    # one output tensor per t-phase of TPC steps; cols = (r, t_in_phase, b)
    out_d = [
        nc.dram_tensor(f"out{ph}", [NVT_C, 128, NC_N, TPC, BS], BF16,
                       kind="ExternalOutput")
        for ph in range(KC)
    ]

    with tile.TileContext(nc) as tc:
        with (
            tc.tile_pool(name="const", bufs=1) as const,
            tc.tile_pool(name="hs", bufs=1) as hsp,
            tc.tile_pool(name="hsall", bufs=3) as hap,
            tc.tile_pool(name="xg", bufs=1) as xgp,
            tc.tile_pool(name="wih", bufs=1) as wihp,
            tc.tile_pool(name="brc", bufs=1) as brcp,
            tc.tile_pool(name="whh", bufs=1) as whhp,
            tc.tile_pool(name="cst", bufs=1) as cstp,
            tc.tile_pool(name="tmp", bufs=1) as tmp,
            tc.tile_pool(name="h8", bufs=2) as h8p,
            tc.tile_pool(name="xgt", bufs=1) as xgtp,
            tc.tile_pool(name="wo", bufs=1) as wo_p,
            tc.tile_pool(name="stg", bufs=3) as stgp,
            tc.tile_pool(name="dram", bufs=1, space="DRAM") as dram,
            tc.tile_pool(name="ps", bufs=2, space="PSUM") as psp,
            tc.tile_pool(name="po", bufs=3, space="PSUM") as pop,
            tc.tile_pool(name="ptr", bufs=1, space="PSUM") as ptrp,
        ):
            xstat_sb = const.tile([128, TB, 4, 128], BF16)
            nc.sync.dma_start(xstat_sb[:], xstat_d[:])
            featst_sb = const.tile([128, 4, 128], BF16)
            nc.sync.dma_start(featst_sb[:], featst_d[:])
            ones_sb = const.tile([1, 128], BF16)
            nc.sync.dma_start(ones_sb[:], ones_d[:])
            id16_sb = const.tile([16, 16], BF16)
            nc.sync.dma_start(id16_sb[:], id16_d[:])
            boutT_sb = const.tile([128, NVT_C], F32)
            nc.sync.dma_start(boutT_sb[:], boutT_d[:])

            hs_sb = hsp.tile([128, 8, T, BS], BF16)  # [hu, hc, t, b]
            xgT = xgp.tile([128, TB, G], BF16)  # [8t*16b, tblk, g] (x SC)
            whh_sb = whhp.tile([128, 4, 2, G], FP8)
            nc.sync.dma_start(whh_sb[:], whh8_d[:])
            c_sb = cstp.tile([16, H2], F32)
            # all W_out tiles as one resident tile, loaded once on the
            # scalar ring so the sync ring stays free for xg staging
            wop_all = wo_p.tile([128, NVT_C, 8, 128], BF16)
            nc.scalar.dma_start(
                wop_all[:], wop_d[:].rearrange("vt p hc j -> p vt hc j")
            )
            hs_all_t = []  # per-AG-chunk gathered hs tiles (ring of 4)
            ag_in = [
                dram.tile([128, 8, TPC, BS], BF16, name=f"agi{k}") for k in range(KC)
            ]
            ag_out = [
                dram.tile([NC_N * 128, 8, TPC, BS], BF16, addr_space="Shared",
                          name=f"ago{k}")
                for k in range(KC)
            ]

            # ---- phase 1: xg GEMM (transposed orientation), W_ih streamed ----
            for cc in range(8):
                ccs = slice(cc * 512, (cc + 1) * 512)
                wih_c = wihp.tile([128, 8, 512], BF16, tag="wih")
                nc.sync.dma_start(wih_c[:], wihT_d[:, :, ccs])
                brow_c = brcp.tile([1, 512], BF16, tag="brc")
                nc.sync.dma_start(brow_c[:], brow_d[:, ccs])
                for tblk in range(TB):
                    px = psp.tile([128, 512], F32, tag="ps")
                    for ec in range(4):
                        nc.tensor.matmul(
                            px[:],
                            xstat_sb[:, tblk, ec],
                            wih_c[:, ec, :],
                            start=(ec == 0),
                            stop=False,
                        )
                    for ec in range(4):
                        nc.tensor.matmul(
                            px[:],
                            featst_sb[:, ec],
                            wih_c[:, 4 + ec, :],
                            start=False,
                            stop=False,
                        )
                    nc.tensor.matmul(
                        px[:], ones_sb[:], brow_c[:], start=False, stop=True
                    )
                    nc.scalar.activation(xgT[:, tblk, ccs], px[:], COPY)

            # ---- phase 2: LSTM (transposed orientation, fp8 recurrence) ----
            # gate quarters: q0=i, q1=f, q2=g, q3=o. Emit o, f, i, g.
            Q_ORDER = [3, 1, 0, 2]
            for t in range(T):
                tblk, p0 = t // 8, (t % 8) * BS
                # stage this step's xg slice down to partition base 0
                # (PE operands require base partition 0/32/64)
                xg_t = xgtp.tile([16, G], BF16, tag="xgt")
                nc.sync.dma_start(xg_t[:], xgT[p0 : p0 + BS, tblk, :])

                t_i = tmp.tile([16, H2], F32, tag="ti")
                t_f = tmp.tile([16, H2], F32, tag="tf")
                t_g = tmp.tile([16, H2], F32, tag="tg")
                t_o = tmp.tile([16, H2], F32, tag="to")
                gate_tmp = {0: t_i, 1: t_f, 2: t_g, 3: t_o}

                for q in Q_ORDER:
                    if t == 0 and q == 1:
                        continue  # f unused at t=0 (c_0 = 0)
                    po_ = psp.tile([16, 1024], F32, tag="ps")
                    h8p_ = h8_prev if t > 0 else None
                    _emit_quarter(nc, po_, id16_sb, xg_t, h8p_, whh_sb, q, t)
                    func = TANH if q == 2 else SIG
                    nc.scalar.activation(
                        gate_tmp[q][:], po_[:], func, scale=1.0 / SC,
                    )

                if t == 0:
                    nc.vector.tensor_mul(c_sb[:], t_i[:], t_g[:])
                else:
                    nc.vector.tensor_mul(t_f[:], t_f[:], c_sb[:])
                    nc.vector.tensor_mul(t_i[:], t_i[:], t_g[:])
                    nc.vector.tensor_add(c_sb[:], t_f[:], t_i[:])
                t_c = tmp.tile([16, H2], F32, tag="tg")  # reuse t_g's slot
                nc.scalar.activation(t_c[:], c_sb[:], TANH)
                hT = tmp.tile([16, H2], BF16, tag="tf")  # reuse t_f's slot
                nc.vector.tensor_mul(hT[:], t_o[:], t_c[:])

                ptr = ptrp.tile([128, 8, BS], BF16, tag="ptr")
                for hc in range(8):
                    nc.tensor.transpose(
                        ptr[:, hc], hT[:, hc * 128 : (hc + 1) * 128], id16_sb[:]
                    )
                nc.vector.tensor_copy(hs_sb[:, :, t, :], ptr[:])
                h8_prev = h8p.tile([128, 8, BS], FP8, tag="h8")
                nc.scalar.activation(h8_prev[:], ptr[:], COPY, scale=HSC)

                # ---- phase 3 (interleaved): chunked AllGather of hs ----
                if t % TPC == TPC - 1:
                    k = t // TPC
                    ts = slice(k * TPC, (k + 1) * TPC)
                    nc.gpsimd.dma_start(out=ag_in[k][:], in_=hs_sb[:, :, ts, :])
                    nc.gpsimd.collective_compute(
                        "AllGather",
                        mybir.AluOpType.bypass,
                        replica_groups=[list(range(NC_N))],
                        ins=[ag_in[k].opt()],
                        outs=[ag_out[k].opt()],
                    )
                    hs_k = hap.tile([128, 8, NC_N, TPC, BS], BF16, tag="ha")
                    hs_all_t.append(hs_k)
                    nc.gpsimd.dma_start(
                        out=hs_k[:],
                        in_=ag_out[k][:].rearrange(
                            "(r p) hc t b -> p hc r t b", p=128
                        ),
                    )

            # ---- phase 4: vocab-sharded projection, pipelined per t-phase ----
            # t-phase ph only needs AllGather chunk ph, so early phases overlap
            # the LSTM; W_out tiles are re-streamed per (phase, vt).
            for ph in range(KC):
                for vt in range(NVT_C):
                    po = pop.tile([128, NC_N, TPC, BS], F32, tag="po")
                    for hc in range(8):
                        nc.tensor.matmul(
                            po[:],
                            wop_all[:, vt, hc],
                            hs_all_t[ph][:, hc],
                            start=(hc == 0),
                            stop=(hc == 7),
                        )
                    st = stgp.tile([128, NC_N, TPC, BS], BF16, tag="st")
                    nc.scalar.activation(
                        st[:], po[:], IDENT, bias=boutT_sb[:, vt : vt + 1]
                    )
                    eng = nc.scalar if ph < 3 else nc.sync
                    eng.dma_start(out_d[ph][vt], st[:])

    nc.compile()
    return nc


def prep_host(features, captions, pad_idx, emb, W_ih, W_hh, b_ih, b_hh, W_out, b_out):
    """Host-side layout prep. Returns (shared dict, per-core list of dicts)."""
    features = np.asarray(features, dtype=np.float32)
    captions = np.asarray(captions).astype(np.int64)
    pad_idx = int(np.asarray(pad_idx))
    emb = np.asarray(emb, dtype=np.float32)
    W_ih = np.asarray(W_ih, dtype=np.float32)
    W_hh = np.asarray(W_hh, dtype=np.float32)
    bsum = np.asarray(b_ih, dtype=np.float32) + np.asarray(b_hh, dtype=np.float32)
    W_out = np.asarray(W_out, dtype=np.float32)
    b_out = np.asarray(b_out, dtype=np.float32)

    # seqtok[t, b]: pad for t=0, captions[b, t-1] for t>=1
    seqtok = np.empty((T, B), np.int64)
    seqtok[0, :] = pad_idx
    seqtok[1:, :] = captions[:, : T - 1].T
    xtok = emb[seqtok]  # [T, B, E]

    # wihT[p, ec, g] = W_ih[g, ec*128+p] * SC
    wihT = np.ascontiguousarray(
        (W_ih * SC).T.reshape(8, 128, G).transpose(1, 0, 2).astype(bf16)
    )
    # whh8[p, j, kt, g] = W_hh[g, (2j+kt)*128+p] * WSC
    whh8 = np.ascontiguousarray(
        (W_hh * WSC).T.reshape(4, 2, 128, G).transpose(2, 0, 1, 3).astype(fp8)
    )
    brow = np.ascontiguousarray((bsum * SC)[None, :].astype(bf16))
    onesrow = np.ones((1, 128), bf16)
    ident16 = np.eye(16, dtype=bf16)

    Wout_pad = np.zeros((VP, H2), np.float32)
    Wout_pad[:V] = W_out
    bout_pad = np.zeros((VP,), np.float32)
    bout_pad[:V] = b_out

    shared = {"wihT": wihT, "whh8": whh8, "brow": brow, "onesrow": onesrow,
              "ident16": ident16}

    per_core = []
    for c in range(NC_N):
        bsl = slice(c * BS, (c + 1) * BS)
        # xstat[p, tblk, ec, ti*16+bl] = xtok[tblk*8+ti, c*16+bl, ec*128+p]
        xs = xtok[:, bsl, :]  # [24, 16, 512]
        xs = xs.reshape(TB, 8, BS, 4, 128)  # [tblk, ti, bl, ec, p]
        xstat = np.ascontiguousarray(
            xs.transpose(4, 0, 3, 1, 2).reshape(128, TB, 4, 8 * BS).astype(bf16)
        )
        # featst[p, ec, ti*16+bl] = features[c*16+bl, ec*128+p]
        f = features[bsl].reshape(BS, 4, 128)  # [bl, ec, p]
        featst = np.ascontiguousarray(
            np.broadcast_to(
                f.transpose(2, 1, 0)[:, :, None, :], (128, 4, 8, BS)
            ).reshape(128, 4, 128).astype(bf16)
        )
        # wop[vt, p(hu), hc, j] = Wout_pad[c*3840 + vt*128 + j, hc*128 + p]
        w = Wout_pad[c * VSH : (c + 1) * VSH].reshape(NVT_C, 128, 8, 128)
        wop = np.ascontiguousarray(w.transpose(0, 3, 2, 1).astype(bf16))
        # boutT[p, vt] = bout_pad[c*3840 + vt*128 + p]
        bT = np.ascontiguousarray(
            bout_pad[c * VSH : (c + 1) * VSH].reshape(NVT_C, 128).T
        )
        per_core.append({"xstat": xstat, "featst": featst, "wop": wop, "boutT": bT})
    return shared, per_core


_NC_CACHE = None


def kernel(**inputs) -> np.ndarray:
    global _NC_CACHE
    if _NC_CACHE is None:
        _NC_CACHE = build_nc()
    nc = _NC_CACHE

    shared, per_core = prep_host(**inputs)
    in_maps = [dict(shared, **pc) for pc in per_core]
    res = run_bass_kernel_spmd(nc, in_maps, core_ids=list(range(NC_N)))

    out = np.empty((B, VP, T), np.float32)
    for c in range(NC_N):
        for ph in range(KC):
            o = np.asarray(res.results[c][f"out{ph}"])  # [30, 128, 8r, TPCt, 16b]
            a = o.astype(np.float32).transpose(2, 4, 0, 1, 3)  # [r, bl, vt, j, tp]
            out[:, c * VSH : (c + 1) * VSH, ph * TPC : (ph + 1) * TPC] = a.reshape(
                B, VSH, TPC
            )
    return out[:, :V, :]
